# revision 131
# baseline (speedup 1.0000x reference)
"""Multi-head attention with relative-position-bias MLP on 8 TRN2 NeuronCores.

Strategy: pure data-parallel over batch (B=8 -> 1 batch element per core, no
collectives). Host-side prep is layout only: per-core transposed x in bf16
(plus a token-reversed copy feeding k/v), transposed bf16 weights, a
replicated bf16 proj bias, and exp() of the 63x63 relative-position bias
table (a 2D-Toeplitz expansion of a tiny MLP; ~7 MFLOP of a 66 GFLOP
problem).

Device algorithm per core (N=1024 tokens, C=768, H=12 heads, D=64):
  v[n,o]   = xRT.T @ wv            (natural layout, token-reversed, + ones
                                    column at h*65+64)
  qT[o,n]  = wq.T @ xT ; kT[o,n] = wk.T @ xRT   (k token-reversed)
  per head h, k-tile t:
     sT = kT_h(t).T @ qT_h         [nk=128, nq=1024] psum
     E  = exp(sT/8)                (ACT, scale folded into exp)
     P  = E * expB_tile            (DVE, bf16, all-SBUF 2x mode;
                                    exp(s+b) = exp(s)*exp(b))
     av[h] += [v_h(t) | 1].T @ P   (PE accumulate; row 64 = colsum)
  outT_h  = av[0:64] * recip(av[64])  (recip on DVE from psum, replicated
                                       across 64 partitions via K=1 matmul)
  final   = outT.T @ pwT + proj_b  (bias added by DVE during psum->sbuf)

Emission interleaves the q/k production and the next head's score matmuls
between each head's score and AV matmuls so the PE never waits on the
ACT(exp) -> DVE(mul) round trip. The k-halves of q/k production for heads
2-7 run in the prologue (k-weights land before q-weights), and all bias-
table DMAs dispatch from the otherwise-idle GPSIMD/SWDGE path so the
shared HWDGE dispatcher never throttles the steady state, which runs
gap-free. Engines: PE 154us busy (85%), DVE 137us, ACT 133us, Pool 55us.

Token reversal trick: bias[h,n,m] depends on grid coords of (n,m) only via
(cy_n - cy_m, cx_n - cx_m). Reversing key/value token order makes the
Toeplitz expansion all-positive-stride: TBLREP_h[p, J] = expG_h[63*(p//32)
+ p%32 + J] (4 plain DMAs per head), and each [128,1024] bias tile is a
strided view of it. The AV reduction over k-tiles is order-invariant.
"""
import sys

import numpy as np

sys.path.insert(0, "/opt/trn_rl_repo")

import concourse.bass as bass  # noqa: E402
import concourse.mybir as mybir  # noqa: E402
import concourse.tile as tile  # noqa: E402
from concourse import bacc  # noqa: E402
from concourse.bass_utils import run_bass_kernel_spmd  # noqa: E402

F32 = mybir.dt.float32
F32R = mybir.dt.float32r
BF16 = mybir.dt.bfloat16
EXP = mybir.ActivationFunctionType.Exp

B, N, C, H, D = 8, 1024, 768, 12, 64
SCALE = float(D) ** -0.5
NT = N // 128   # 8 token tiles
CT = C // 128   # 6 channel tiles
TBLW = 3781     # TBLREP width (padded so 2016-wide views stay in range)
TW = 4001       # DRAM table width per head (>= 220 + TBLW, zero-padded)


def _build_graph():
    nc = bacc.Bacc("TRN2", target_bir_lowering=False, debug=False,
                   enable_asserts=False, num_devices=B)
    xrwk_d = nc.dram_tensor("xrwk", [C, N + C], BF16, kind="ExternalInput")
    wvq_d = nc.dram_tensor("wvq", [C, 2 * C], BF16, kind="ExternalInput")
    wproj_d = nc.dram_tensor("proj_wT", [C, C], BF16, kind="ExternalInput")
    pbrep_d = nc.dram_tensor("proj_b_rep", [128, C], F32, kind="ExternalInput")
    tbl_d = nc.dram_tensor("rpb_tbl", [H, TW], BF16, kind="ExternalInput")
    out_d = nc.dram_tensor("out", [N, C], BF16, kind="ExternalOutput")

    with tile.TileContext(nc) as tc:
        _kern(tc, nc, xrwk_d, wvq_d, wproj_d, pbrep_d, tbl_d, out_d)
    nc.compile()
    return nc


def _kern(tc, nc, xrwk_d, wvq_d, wproj_d, pbrep_d, tbl_d, out_d):
    from contextlib import ExitStack

    with ExitStack() as es:
        persist = es.enter_context(tc.tile_pool(name="persist", bufs=1))
        # qT tiles 0..5, kT tiles 6..11; [o-part, n-free], bf16
        qk_sb = [persist.tile([128, N], BF16, tag=f"qk{i}", name=f"qk{i}")
                 for i in range(12)]
        # v (token-reversed) head-strided with ones column at h*65+64
        vaug = [persist.tile([128, H * 65], BF16, tag=f"va{i}", name=f"va{i}")
                for i in range(NT)]
        # attention output transposed [c, n], c = h*64+d, bf16 (proj lhsT)
        outT = [persist.tile([128, N], BF16, tag=f"ot{i}", name=f"ot{i}")
                for i in range(CT)]
        ones_f = persist.tile([128, 64], F32, tag="onesf")
        nc.vector.memset(ones_f[:], 1.0)
        ones_r = persist.tile([128, 64], F32R, tag="onesr")
        nc.vector.tensor_copy(ones_r[:], ones_f[:])
        onescol = persist.tile([128, H], BF16, tag="onescol")
        nc.gpsimd.memset(onescol[:], 1.0)
        for t in range(NT):
            va_v = vaug[t][:].rearrange("p (h e) -> p h e", e=65)
            nc.gpsimd.tensor_copy(va_v[:, :, 64:65], onescol[:].unsqueeze(-1))

        # weight/x loads (bf16); dispatched from SP + ACT queues (HWDGE)
        ld = es.enter_context(tc.tile_pool(name="ld", bufs=1))
        xrwk = [ld.tile([128, N + C], BF16, tag=f"xk{i}", name=f"xk{i}")
                for i in range(CT)]
        wvq = [ld.tile([128, 2 * C], BF16, tag=f"wvq{i}", name=f"wvq{i}")
               for i in range(CT)]
        xRT = [t[:, 0:N] for t in xrwk]
        wv = [t[:, 0:C] for t in wvq]
        pwT = [persist.tile([128, C], BF16, tag=f"pw{i}", name=f"pw{i}")
               for i in range(CT)]
        pbrep = persist.tile([128, C], F32, tag="pbrep")
        # xRT + k weights first (the prologue's score pipeline needs only
        # those); q weights next; wv via SWDGE in parallel; proj weights
        # deferred into the head loop.
        for i in range(CT):
            eng = nc.sync if i % 2 == 0 else nc.scalar
            eng.dma_start(xrwk[i][:],
                          xrwk_d.ap()[i * 128:(i + 1) * 128, :])
        for i in range(CT):
            eng = nc.scalar if i % 2 == 0 else nc.sync
            eng.dma_start(wvq[i][:],
                          wvq_d.ap()[i * 128:(i + 1) * 128, :])

        # PSUM: scores + all transients 3 x [128,1024] (12KB/partition);
        # AV accumulators 2 x [65,512] (4KB). Total 16KB = all 8 banks.
        bigps = es.enter_context(tc.tile_pool(name="bigps", bufs=2,
                                              space="PSUM"))
        avps = es.enter_context(tc.tile_pool(name="avps", bufs=2,
                                             space="PSUM"))
        smps = es.enter_context(tc.tile_pool(name="smps", bufs=2,
                                             space="PSUM"))
        tblp = es.enter_context(tc.tile_pool(name="tblp", bufs=3))
        ep = es.enter_context(tc.tile_pool(name="expp", bufs=8))
        pp = es.enter_context(tc.tile_pool(name="phat", bufs=8))
        tmpp = es.enter_context(tc.tile_pool(name="tmp", bufs=4))

        def emit_v_group(t, vc):
            ps = smps.tile([128, 384], F32, tag="sm", name=f"vps{t}_{vc}")
            for kt in range(CT):
                nc.tensor.matmul(
                    ps[:], xrwk[kt][:, t * 128:(t + 1) * 128],
                    wvq[kt][:, vc * 384:(vc + 1) * 384],
                    start=(kt == 0), stop=(kt == CT - 1))
            va_v = vaug[t][:].rearrange("p (h e) -> p h e", e=65)
            ps_v = ps[:].rearrange("p (h d) -> p h d", d=64)
            with nc.allow_low_precision(reason="v rounded to bf16"):
                if vc == 0:
                    nc.scalar.activation(va_v[:, 0:6, 0:64], ps_v,
                                         mybir.ActivationFunctionType.Copy)
                else:
                    nc.vector.tensor_copy(va_v[:, 6:12, 0:64], ps_v)

        def emit_qk_group(ot, c):
            """One 6-matmul accumulation producing qk_sb[ot][:, c*512:...]."""
            oo = (ot % 6) * 128
            ps = smps.tile([128, 512], F32, tag="sm", name=f"qkps{ot}_{c}")
            for kt in range(CT):
                w = (xrwk[kt][:, N + oo:N + oo + 128] if ot >= 6
                     else wvq[kt][:, C + oo:C + oo + 128])
                nc.tensor.matmul(
                    ps[:], w, xRT[kt][:, c * 512:(c + 1) * 512],
                    start=(kt == 0), stop=(kt == CT - 1))
            with nc.allow_low_precision(reason="q/k rounded to bf16"):
                nc.vector.tensor_copy(qk_sb[ot][:, c * 512:(c + 1) * 512],
                                      ps[:])

        def emit_tbl_dma(h, tblt):
            for blk in range(4):
                nc.gpsimd.dma_start(
                    tblt[blk * 32:(blk + 1) * 32, :],
                    bass.AP(tbl_d, h * TW + 63 * blk, [[1, 32], [1, TBLW]]))

        def emit_s(h, t, pss):
            """Score matmuls for (head h, k-tile t) into [128,1024] psum."""
            j = h // 2
            qh = qk_sb[j][(h % 2) * 64:(h % 2) * 64 + 64, :]
            kh = qk_sb[6 + j][(h % 2) * 64:(h % 2) * 64 + 64, :]
            for c in range(2):
                nc.tensor.matmul(
                    pss[:, c * 512:(c + 1) * 512],
                    kh[:, t * 128:(t + 1) * 128],
                    qh[:, c * 512:(c + 1) * 512],
                    start=True, stop=True)

        def emit_expmul(h, t, pss, tblt, avs):
            """exp (ACT) + bias multiply (DVE/Pool) + AV matmuls (PE)."""
            ee = ep.tile([128, 1024], BF16, tag="ee", name=f"ee{h}_{t}")
            nc.scalar.activation(ee[:], pss[:], EXP, scale=SCALE)
            tap = tblt[:]
            tv = bass.AP(tap.tensor, tap.offset + 1984 + 252 * t,
                         [[TBLW, 128], [-1008, 2], [-63, 16], [-1, 32]])
            ph = pp.tile([128, 1024], BF16, tag="ph", name=f"ph{h}_{t}")
            pv = ph[:].rearrange("p (c a b) -> p c a b", c=2, b=32)
            ev = ee[:].rearrange("p (c a b) -> p c a b", c=2, b=32)
            nc.vector.tensor_mul(pv, ev, tv)
            for c in range(2):
                nc.tensor.matmul(
                    avs[c][:], vaug[t][:, h * 65:(h + 1) * 65],
                    ph[:, c * 512:(c + 1) * 512],
                    start=(t == 0), stop=(t == NT - 1))

        def emit_recip(h, avs):
            """Drain AV psum to SBUF (DVE/ACT split) and recip the colsum
            row; the out-mul then has only one PSUM operand (rep)."""
            avsbs, rsbs = [], []
            for c in range(2):
                avsb = tmpp.tile([65, 512], F32, tag="avsb",
                                 name=f"avsb{h}{c}")
                nc.scalar.activation(avsb[:], avs[c][:],
                                     mybir.ActivationFunctionType.Copy)
                rsb = tmpp.tile([128, 512], F32R, tag="rsb",
                                name=f"rsb{h}{c}")
                with nc.allow_low_precision(
                        reason="softmax recip rounded to f32r"):
                    nc.vector.reciprocal(rsb[64:65, :], avsb[64:65, :])
                avsbs.append(avsb)
                rsbs.append(rsb)
            return avsbs, rsbs

        def emit_divide(h, avsbs, rsbs):
            """replicate recip via K=1 matmul, multiply, store to outT.
            The free dim carries reversed tokens; writes flip it back."""
            row0 = (h % 2) * 64
            tmp = None
            if h % 2 == 1:
                tmp = tmpp.tile([64, N], BF16, tag="tmo", name=f"tmo{h}")
            for c in range(2):
                rep = smps.tile([64, 512], F32, tag="sm", name=f"rep{h}{c}")
                nc.tensor.matmul(rep[:], ones_r[64:65, 0:64],
                                 rsbs[c][64:65, :], start=True, stop=True)
                rsl = slice(N - 1 - 512 * c, None if c == 1 else 511, -1)
                with nc.allow_low_precision(reason="attn out bf16"):
                    if h % 2 == 0:
                        dst = outT[h // 2][row0:row0 + 64, rsl]
                        nc.vector.tensor_mul(dst, avsbs[c][0:64, :], rep[:])
                    else:
                        nc.vector.tensor_mul(tmp[:, rsl], avsbs[c][0:64, :],
                                             rep[:])
                        if c == 1:
                            nc.sync.dma_start(
                                outT[h // 2][row0:row0 + 64, :], tmp[:])

        # ---- pipelined attention: q/k production + per-head S/exp/mul/AV
        # interleaved so the PE never waits on the ACT->DVE round trip ----
        emit_qk_group(6, 0)
        emit_qk_group(6, 1)
        emit_qk_group(7, 0)
        emit_qk_group(7, 1)
        emit_qk_group(8, 0)
        emit_qk_group(8, 1)
        emit_qk_group(9, 0)
        emit_qk_group(9, 1)
        emit_qk_group(0, 0)
        emit_qk_group(0, 1)

        tbls = {}
        tbls[0] = tblp.tile([128, TBLW], BF16, tag="tbl", name="tbl0")
        emit_tbl_dma(0, tbls[0])
        tbls[1] = tblp.tile([128, TBLW], BF16, tag="tbl", name="tbl1")
        emit_tbl_dma(1, tbls[1])

        pss_cur = [None] * NT
        pss_nxt = [None] * NT

        # prologue: scores for head 0
        for t in range(NT):
            pss_cur[t] = bigps.tile([128, 1024], F32, tag="s",
                                    name=f"s0_{t}")
            emit_s(0, t, pss_cur[t])

        LAG = 2  # exp/mul/AV trail the next head's score matmuls by 2 tiles
        prev = None  # (h-1, avs, rsbs) awaiting divide
        for h in range(H):
            avs = [avps.tile([65, 512], F32, tag="av", name=f"av{h}_{c}")
                   for c in range(2)]
            if h + 2 < H:
                tbls[h + 2] = tblp.tile([128, TBLW], BF16, tag="tbl",
                                        name=f"tbl{h + 2}")
                emit_tbl_dma(h + 2, tbls[h + 2])
            if h == 1:
                for i in range(CT):
                    eng = nc.sync if i % 2 == 0 else nc.scalar
                    eng.dma_start(pwT[i][:],
                                  wproj_d.ap()[i * 128:(i + 1) * 128, :])
                nc.sync.dma_start(pbrep[:], pbrep_d.ap()[:, :])
            for step in range(NT + LAG):
                if step < NT and h + 1 < H:
                    pss_nxt[step] = bigps.tile([128, 1024], F32, tag="s",
                                               name=f"s{h + 1}_{step}")
                    emit_s(h + 1, step, pss_nxt[step])
                if step == 8 and prev is not None:
                    emit_divide(*prev)
                    prev = None
                if step >= LAG:
                    emit_expmul(h, step - LAG, pss_cur[step - LAG],
                                tbls[h], avs)
                # v production rides head 0's steps as PE filler
                if h == 0 and step < NT:
                    emit_v_group(step, 0)
                    emit_v_group(step, 1)
                # q/k production for j=1 in loop 0's drain steps, j>=2
                # spread over later even loops; always emitted before the
                # S matmuls of the heads that read them
                if h == 0 and step in (8, 9):
                    emit_qk_group(1, 0 if step == 8 else 1)
                if h in (2, 4) and step in (0, 2):
                    emit_qk_group(h // 2 + 1, (step // 2) % 2)
                if h % 2 == 0 and h >= 6 and h // 2 + 1 < 6 and \
                        step in (0, 2, 4, 6):
                    j = h // 2 + 1
                    ot = j if step < 4 else 6 + j
                    emit_qk_group(ot, (step // 2) % 2)
            avsbs, rsbs = emit_recip(h, avs)
            prev = (h, avsbs, rsbs)
            pss_cur, pss_nxt = pss_nxt, [None] * NT
        emit_divide(*prev)

        # ---------------- proj ----------------
        fsb = es.enter_context(tc.tile_pool(name="fsb", bufs=4))
        for t in range(NT):
            f = fsb.tile([128, C], BF16, tag="f", name=f"f{t}")
            for pc in range(2):
                ps = smps.tile([128, 384], F32, tag="sm", name=f"pj{t}{pc}")
                for kt in range(CT):
                    nc.tensor.matmul(
                        ps[:], outT[kt][:, t * 128:(t + 1) * 128],
                        pwT[kt][:, pc * 384:(pc + 1) * 384],
                        start=(kt == 0), stop=(kt == CT - 1))
                # bias added during the psum->sbuf move (DVE); each half
                # is written out as soon as its add completes
                with nc.allow_low_precision(reason="output rounded to bf16"):
                    nc.vector.tensor_add(f[:, pc * 384:(pc + 1) * 384],
                                         ps[:],
                                         pbrep[:, pc * 384:(pc + 1) * 384])
                eng = (nc.sync, nc.scalar, nc.gpsimd)[(t * 2 + pc) % 3]
                eng.dma_start(
                    out_d.ap()[t * 128:(t + 1) * 128,
                               pc * 384:(pc + 1) * 384],
                    f[:, pc * 384:(pc + 1) * 384])


_GRAPH = None


def _graph():
    global _GRAPH
    if _GRAPH is None:
        _GRAPH = _build_graph()
    return _GRAPH


def _host_prep(x, qkv_w, proj_w, proj_b, rpb_w1, rpb_b1, rpb_w2, rpb_b2):
    """Numpy layout prep + exp of the 63x63 bias table (7 MFLOP)."""
    import ml_dtypes
    a = np.arange(63, dtype=np.float32) - 31.0
    rel_y = np.broadcast_to(a[:, None], (63, 63))
    rel_x = np.broadcast_to(a[None, :], (63, 63))
    rel = np.stack([rel_x, rel_y], -1).reshape(-1, 2)           # [3969, 2]
    hdn = np.maximum(rel @ rpb_w1.T + rpb_b1, 0.0)
    gtbl = (hdn @ rpb_w2.T + rpb_b2).T.astype(np.float32)       # [12, 3969]
    gtbl = np.exp(gtbl, dtype=np.float32)                       # exp(bias)
    gpad = np.zeros((H, TW), np.float32)
    gpad[:, :3969] = gtbl
    gpad = gpad.astype(ml_dtypes.bfloat16)

    bf = ml_dtypes.bfloat16
    wqkvT = np.ascontiguousarray(qkv_w.T).astype(np.float32)    # [768, 2304]
    wvq = np.ascontiguousarray(np.concatenate(
        [wqkvT[:, 2 * C:3 * C], wqkvT[:, 0:C]], axis=1)).astype(bf)
    wkT = wqkvT[:, C:2 * C]
    wprojT = np.ascontiguousarray(proj_w.T).astype(bf)          # [768, 768]
    pbrep = np.ascontiguousarray(
        np.broadcast_to(proj_b.astype(np.float32), (128, C)))
    shared = {"wvq": wvq, "proj_wT": wprojT, "proj_b_rep": pbrep,
              "rpb_tbl": gpad}
    in_maps = []
    for i in range(B):
        m = dict(shared)
        m["xrwk"] = np.ascontiguousarray(np.concatenate(
            [x[i][::-1].T, wkT], axis=1)).astype(bf)
        in_maps.append(m)
    return in_maps


def kernel(x, qkv_w, proj_w, proj_b, rpb_w1, rpb_b1, rpb_w2, rpb_b2,
           _trace=False, _tmpdir=None):
    in_maps = _host_prep(np.asarray(x), np.asarray(qkv_w), np.asarray(proj_w),
                         np.asarray(proj_b), np.asarray(rpb_w1),
                         np.asarray(rpb_b1), np.asarray(rpb_w2),
                         np.asarray(rpb_b2))
    nc = _graph()
    res = run_bass_kernel_spmd(nc, in_maps, core_ids=list(range(B)),
                               trace=_trace, tmpdir=_tmpdir)
    out = np.stack([np.asarray(res.results[i]["out"], dtype=np.float32)
                    for i in range(B)])
    if _trace:
        kernel._last_results = res
    return out


# revision 134
# speedup vs baseline: 1.0018x; 1.0018x over previous
"""Multi-head attention with relative-position-bias MLP on 8 TRN2 NeuronCores.

Strategy: pure data-parallel over batch (B=8 -> 1 batch element per core, no
collectives). Host-side prep is layout only: per-core transposed x in bf16
(plus a token-reversed copy feeding k/v), transposed bf16 weights, a
replicated bf16 proj bias, and exp() of the 63x63 relative-position bias
table (a 2D-Toeplitz expansion of a tiny MLP; ~7 MFLOP of a 66 GFLOP
problem).

Device algorithm per core (N=1024 tokens, C=768, H=12 heads, D=64):
  v[n,o]   = xRT.T @ wv            (natural layout, token-reversed, + ones
                                    column at h*65+64)
  qT[o,n]  = wq.T @ xT ; kT[o,n] = wk.T @ xRT   (k token-reversed)
  per head h, k-tile t:
     sT = kT_h(t).T @ qT_h         [nk=128, nq=1024] psum
     E  = exp(sT/8)                (ACT, scale folded into exp)
     P  = E * expB_tile            (DVE, bf16, all-SBUF 2x mode;
                                    exp(s+b) = exp(s)*exp(b))
     av[h] += [v_h(t) | 1].T @ P   (PE accumulate; row 64 = colsum)
  outT_h  = av[0:64] * recip(av[64])  (recip on DVE from psum, replicated
                                       across 64 partitions via K=1 matmul)
  final   = outT.T @ pwT + proj_b  (bias added by DVE during psum->sbuf)

Emission interleaves the q/k production and the next head's score matmuls
between each head's score and AV matmuls so the PE never waits on the
ACT(exp) -> DVE(mul) round trip. The k-halves of q/k production for heads
2-7 run in the prologue (k-weights land before q-weights), and all bias-
table DMAs dispatch from the otherwise-idle GPSIMD/SWDGE path so the
shared HWDGE dispatcher never throttles the steady state, which runs
gap-free. Engines: PE 154us busy (85%), DVE 137us, ACT 133us, Pool 55us.

Token reversal trick: bias[h,n,m] depends on grid coords of (n,m) only via
(cy_n - cy_m, cx_n - cx_m). Reversing key/value token order makes the
Toeplitz expansion all-positive-stride: TBLREP_h[p, J] = expG_h[63*(p//32)
+ p%32 + J] (4 plain DMAs per head), and each [128,1024] bias tile is a
strided view of it. The AV reduction over k-tiles is order-invariant.
"""
import sys

import numpy as np

sys.path.insert(0, "/opt/trn_rl_repo")

import concourse.bass as bass  # noqa: E402
import concourse.mybir as mybir  # noqa: E402
import concourse.tile as tile  # noqa: E402
from concourse import bacc  # noqa: E402
from concourse.bass_utils import run_bass_kernel_spmd  # noqa: E402

F32 = mybir.dt.float32
F32R = mybir.dt.float32r
BF16 = mybir.dt.bfloat16
EXP = mybir.ActivationFunctionType.Exp

B, N, C, H, D = 8, 1024, 768, 12, 64
SCALE = float(D) ** -0.5
NT = N // 128   # 8 token tiles
CT = C // 128   # 6 channel tiles
TBLW = 3781     # TBLREP width (padded so 2016-wide views stay in range)
TW = 4001       # DRAM table width per head (>= 220 + TBLW, zero-padded)


def _build_graph():
    nc = bacc.Bacc("TRN2", target_bir_lowering=False, debug=False,
                   enable_asserts=False, num_devices=B)
    xrwk_d = nc.dram_tensor("xrwk", [C, N + C], BF16, kind="ExternalInput")
    wvq_d = nc.dram_tensor("wvq", [C, 2 * C], BF16, kind="ExternalInput")
    wproj_d = nc.dram_tensor("proj_wT", [C, C], BF16, kind="ExternalInput")
    pbrep_d = nc.dram_tensor("proj_b_rep", [128, C], F32, kind="ExternalInput")
    tbl_d = nc.dram_tensor("rpb_tbl", [H, TW], BF16, kind="ExternalInput")
    out_d = nc.dram_tensor("out", [N, C], BF16, kind="ExternalOutput")

    with tile.TileContext(nc) as tc:
        _kern(tc, nc, xrwk_d, wvq_d, wproj_d, pbrep_d, tbl_d, out_d)
    nc.compile()
    return nc


def _kern(tc, nc, xrwk_d, wvq_d, wproj_d, pbrep_d, tbl_d, out_d):
    from contextlib import ExitStack

    with ExitStack() as es:
        persist = es.enter_context(tc.tile_pool(name="persist", bufs=1))
        # qT tiles 0..5, kT tiles 6..11; [o-part, n-free], bf16
        qk_sb = [persist.tile([128, N], BF16, tag=f"qk{i}", name=f"qk{i}")
                 for i in range(12)]
        # v (token-reversed) head-strided with ones column at h*65+64
        vaug = [persist.tile([128, H * 65], BF16, tag=f"va{i}", name=f"va{i}")
                for i in range(NT)]
        # attention output transposed [c, n], c = h*64+d, bf16 (proj lhsT)
        outT = [persist.tile([128, N], BF16, tag=f"ot{i}", name=f"ot{i}")
                for i in range(CT)]
        ones_f = persist.tile([128, 64], F32, tag="onesf")
        nc.vector.memset(ones_f[:], 1.0)
        ones_r = persist.tile([128, 64], F32R, tag="onesr")
        nc.vector.tensor_copy(ones_r[:], ones_f[:])
        onescol = persist.tile([128, H], BF16, tag="onescol")
        nc.gpsimd.memset(onescol[:], 1.0)
        for t in range(NT):
            va_v = vaug[t][:].rearrange("p (h e) -> p h e", e=65)
            nc.gpsimd.tensor_copy(va_v[:, :, 64:65], onescol[:].unsqueeze(-1))

        # weight/x loads (bf16); dispatched from SP + ACT queues (HWDGE)
        ld = es.enter_context(tc.tile_pool(name="ld", bufs=1))
        xrwk = [ld.tile([128, N + C], BF16, tag=f"xk{i}", name=f"xk{i}")
                for i in range(CT)]
        wvq = [ld.tile([128, 2 * C], BF16, tag=f"wvq{i}", name=f"wvq{i}")
               for i in range(CT)]
        xRT = [t[:, 0:N] for t in xrwk]
        wv = [t[:, 0:C] for t in wvq]
        pwT = [persist.tile([128, C], BF16, tag=f"pw{i}", name=f"pw{i}")
               for i in range(CT)]
        pbrep = persist.tile([128, C], F32, tag="pbrep")
        # xRT + k weights first (the prologue's score pipeline needs only
        # those); q weights next; wv via SWDGE in parallel; proj weights
        # deferred into the head loop.
        for i in range(CT):
            eng = nc.sync if i % 2 == 0 else nc.scalar
            eng.dma_start(xrwk[i][:],
                          xrwk_d.ap()[i * 128:(i + 1) * 128, :])
        for i in range(CT):
            eng = nc.scalar if i % 2 == 0 else nc.sync
            eng.dma_start(wvq[i][:],
                          wvq_d.ap()[i * 128:(i + 1) * 128, :])

        # PSUM: scores + all transients 3 x [128,1024] (12KB/partition);
        # AV accumulators 2 x [65,512] (4KB). Total 16KB = all 8 banks.
        bigps = es.enter_context(tc.tile_pool(name="bigps", bufs=2,
                                              space="PSUM"))
        avps = es.enter_context(tc.tile_pool(name="avps", bufs=2,
                                             space="PSUM"))
        smps = es.enter_context(tc.tile_pool(name="smps", bufs=2,
                                             space="PSUM"))
        tblp = es.enter_context(tc.tile_pool(name="tblp", bufs=3))
        ep = es.enter_context(tc.tile_pool(name="expp", bufs=8))
        pp = es.enter_context(tc.tile_pool(name="phat", bufs=8))
        tmpp = es.enter_context(tc.tile_pool(name="tmp", bufs=4))

        def emit_v_group(t, vc):
            ps = smps.tile([128, 384], F32, tag="sm", name=f"vps{t}_{vc}")
            for kt in range(CT):
                nc.tensor.matmul(
                    ps[:], xrwk[kt][:, t * 128:(t + 1) * 128],
                    wvq[kt][:, vc * 384:(vc + 1) * 384],
                    start=(kt == 0), stop=(kt == CT - 1))
            va_v = vaug[t][:].rearrange("p (h e) -> p h e", e=65)
            ps_v = ps[:].rearrange("p (h d) -> p h d", d=64)
            with nc.allow_low_precision(reason="v rounded to bf16"):
                if vc == 0:
                    nc.scalar.activation(va_v[:, 0:6, 0:64], ps_v,
                                         mybir.ActivationFunctionType.Copy)
                else:
                    nc.vector.tensor_copy(va_v[:, 6:12, 0:64], ps_v)

        def emit_qk_group(ot, c):
            """One 6-matmul accumulation producing qk_sb[ot][:, c*512:...]."""
            oo = (ot % 6) * 128
            ps = smps.tile([128, 512], F32, tag="sm", name=f"qkps{ot}_{c}")
            for kt in range(CT):
                w = (xrwk[kt][:, N + oo:N + oo + 128] if ot >= 6
                     else wvq[kt][:, C + oo:C + oo + 128])
                nc.tensor.matmul(
                    ps[:], w, xRT[kt][:, c * 512:(c + 1) * 512],
                    start=(kt == 0), stop=(kt == CT - 1))
            with nc.allow_low_precision(reason="q/k rounded to bf16"):
                nc.vector.tensor_copy(qk_sb[ot][:, c * 512:(c + 1) * 512],
                                      ps[:])

        def emit_tbl_dma(h, tblt):
            for blk in range(4):
                nc.gpsimd.dma_start(
                    tblt[blk * 32:(blk + 1) * 32, :],
                    bass.AP(tbl_d, h * TW + 63 * blk, [[1, 32], [1, TBLW]]))

        def emit_s(h, t, pss):
            """Score matmuls for (head h, k-tile t) into [128,1024] psum."""
            j = h // 2
            qh = qk_sb[j][(h % 2) * 64:(h % 2) * 64 + 64, :]
            kh = qk_sb[6 + j][(h % 2) * 64:(h % 2) * 64 + 64, :]
            for c in range(2):
                nc.tensor.matmul(
                    pss[:, c * 512:(c + 1) * 512],
                    kh[:, t * 128:(t + 1) * 128],
                    qh[:, c * 512:(c + 1) * 512],
                    start=True, stop=True)

        def emit_expmul(h, t, pss, tblt, avs):
            """exp (ACT) + bias multiply (DVE/Pool) + AV matmuls (PE)."""
            ee = ep.tile([128, 1024], BF16, tag="ee", name=f"ee{h}_{t}")
            nc.scalar.activation(ee[:], pss[:], EXP, scale=SCALE)
            tap = tblt[:]
            tv = bass.AP(tap.tensor, tap.offset + 1984 + 252 * t,
                         [[TBLW, 128], [-1008, 2], [-63, 16], [-1, 32]])
            ph = pp.tile([128, 1024], BF16, tag="ph", name=f"ph{h}_{t}")
            pv = ph[:].rearrange("p (c a b) -> p c a b", c=2, b=32)
            ev = ee[:].rearrange("p (c a b) -> p c a b", c=2, b=32)
            nc.vector.tensor_mul(pv, ev, tv)
            for c in range(2):
                nc.tensor.matmul(
                    avs[c][:], vaug[t][:, h * 65:(h + 1) * 65],
                    ph[:, c * 512:(c + 1) * 512],
                    start=(t == 0), stop=(t == NT - 1))

        def emit_recip(h, avs):
            """Drain AV psum to SBUF (DVE/ACT split) and recip the colsum
            row; the out-mul then has only one PSUM operand (rep)."""
            avsbs, rsbs = [], []
            for c in range(2):
                avsb = tmpp.tile([65, 512], F32, tag="avsb",
                                 name=f"avsb{h}{c}")
                nc.scalar.activation(avsb[:], avs[c][:],
                                     mybir.ActivationFunctionType.Copy)
                rsb = tmpp.tile([128, 512], F32R, tag="rsb",
                                name=f"rsb{h}{c}")
                with nc.allow_low_precision(
                        reason="softmax recip rounded to f32r"):
                    nc.vector.reciprocal(rsb[64:65, :], avsb[64:65, :])
                avsbs.append(avsb)
                rsbs.append(rsb)
            return avsbs, rsbs

        def emit_divide(h, avsbs, rsbs):
            """replicate recip via K=1 matmul, multiply, store to outT.
            The free dim carries reversed tokens; writes flip it back."""
            row0 = (h % 2) * 64
            tmp = None
            if h % 2 == 1:
                tmp = tmpp.tile([64, N], BF16, tag="tmo", name=f"tmo{h}")
            for c in range(2):
                rep = smps.tile([64, 512], F32, tag="sm", name=f"rep{h}{c}")
                nc.tensor.matmul(rep[:], ones_r[64:65, 0:64],
                                 rsbs[c][64:65, :], start=True, stop=True)
                rsl = slice(N - 1 - 512 * c, None if c == 1 else 511, -1)
                with nc.allow_low_precision(reason="attn out bf16"):
                    if h % 2 == 0:
                        dst = outT[h // 2][row0:row0 + 64, rsl]
                        nc.vector.tensor_mul(dst, avsbs[c][0:64, :], rep[:])
                    else:
                        nc.vector.tensor_mul(tmp[:, rsl], avsbs[c][0:64, :],
                                             rep[:])
                        if c == 1:
                            nc.sync.dma_start(
                                outT[h // 2][row0:row0 + 64, :], tmp[:])

        # ---- pipelined attention: q/k production + per-head S/exp/mul/AV
        # interleaved so the PE never waits on the ACT->DVE round trip ----
        emit_qk_group(6, 0)
        emit_qk_group(6, 1)
        emit_qk_group(7, 0)
        emit_qk_group(7, 1)
        emit_qk_group(8, 0)
        emit_qk_group(8, 1)
        emit_qk_group(9, 0)
        emit_qk_group(9, 1)
        emit_qk_group(0, 0)
        emit_qk_group(0, 1)

        tbls = {}
        tbls[0] = tblp.tile([128, TBLW], BF16, tag="tbl", name="tbl0")
        emit_tbl_dma(0, tbls[0])
        tbls[1] = tblp.tile([128, TBLW], BF16, tag="tbl", name="tbl1")
        emit_tbl_dma(1, tbls[1])

        pss_cur = [None] * NT
        pss_nxt = [None] * NT

        # prologue: scores for head 0
        for t in range(NT):
            pss_cur[t] = bigps.tile([128, 1024], F32, tag="s",
                                    name=f"s0_{t}")
            emit_s(0, t, pss_cur[t])

        LAG = 3  # exp/mul/AV trail the next head's score matmuls by 2 tiles
        prev = None  # (h-1, avs, rsbs) awaiting divide
        for h in range(H):
            avs = [avps.tile([65, 512], F32, tag="av", name=f"av{h}_{c}")
                   for c in range(2)]
            if h + 2 < H:
                tbls[h + 2] = tblp.tile([128, TBLW], BF16, tag="tbl",
                                        name=f"tbl{h + 2}")
                emit_tbl_dma(h + 2, tbls[h + 2])
            if h == 1:
                for i in range(CT):
                    eng = nc.sync if i % 2 == 0 else nc.scalar
                    eng.dma_start(pwT[i][:],
                                  wproj_d.ap()[i * 128:(i + 1) * 128, :])
                nc.sync.dma_start(pbrep[:], pbrep_d.ap()[:, :])
            for step in range(NT + LAG):
                if step < NT and h + 1 < H:
                    pss_nxt[step] = bigps.tile([128, 1024], F32, tag="s",
                                               name=f"s{h + 1}_{step}")
                    emit_s(h + 1, step, pss_nxt[step])
                if step == 8 and prev is not None:
                    emit_divide(*prev)
                    prev = None
                if step >= LAG:
                    emit_expmul(h, step - LAG, pss_cur[step - LAG],
                                tbls[h], avs)
                # v production rides head 0's steps as PE filler
                if h == 0 and step < NT:
                    emit_v_group(step, 0)
                    emit_v_group(step, 1)
                # q/k production for j=1 in loop 0's drain steps, j>=2
                # spread over later even loops; always emitted before the
                # S matmuls of the heads that read them
                if h == 0 and step in (8, 9):
                    emit_qk_group(1, 0 if step == 8 else 1)
                if h in (2, 4) and step in (0, 2):
                    emit_qk_group(h // 2 + 1, (step // 2) % 2)
                if h % 2 == 0 and h >= 6 and h // 2 + 1 < 6 and \
                        step in (0, 2, 4, 6):
                    j = h // 2 + 1
                    ot = j if step < 4 else 6 + j
                    emit_qk_group(ot, (step // 2) % 2)
            avsbs, rsbs = emit_recip(h, avs)
            prev = (h, avsbs, rsbs)
            pss_cur, pss_nxt = pss_nxt, [None] * NT
        emit_divide(*prev)

        # ---------------- proj ----------------
        fsb = es.enter_context(tc.tile_pool(name="fsb", bufs=4))
        for t in range(NT):
            f = fsb.tile([128, C], BF16, tag="f", name=f"f{t}")
            for pc in range(2):
                ps = smps.tile([128, 384], F32, tag="sm", name=f"pj{t}{pc}")
                for kt in range(CT):
                    nc.tensor.matmul(
                        ps[:], outT[kt][:, t * 128:(t + 1) * 128],
                        pwT[kt][:, pc * 384:(pc + 1) * 384],
                        start=(kt == 0), stop=(kt == CT - 1))
                # bias added during the psum->sbuf move (DVE); each half
                # is written out as soon as its add completes
                with nc.allow_low_precision(reason="output rounded to bf16"):
                    nc.vector.tensor_add(f[:, pc * 384:(pc + 1) * 384],
                                         ps[:],
                                         pbrep[:, pc * 384:(pc + 1) * 384])
                eng = (nc.sync, nc.scalar, nc.gpsimd)[(t * 2 + pc) % 3]
                eng.dma_start(
                    out_d.ap()[t * 128:(t + 1) * 128,
                               pc * 384:(pc + 1) * 384],
                    f[:, pc * 384:(pc + 1) * 384])


_GRAPH = None


def _graph():
    global _GRAPH
    if _GRAPH is None:
        _GRAPH = _build_graph()
    return _GRAPH


def _host_prep(x, qkv_w, proj_w, proj_b, rpb_w1, rpb_b1, rpb_w2, rpb_b2):
    """Numpy layout prep + exp of the 63x63 bias table (7 MFLOP)."""
    import ml_dtypes
    a = np.arange(63, dtype=np.float32) - 31.0
    rel_y = np.broadcast_to(a[:, None], (63, 63))
    rel_x = np.broadcast_to(a[None, :], (63, 63))
    rel = np.stack([rel_x, rel_y], -1).reshape(-1, 2)           # [3969, 2]
    hdn = np.maximum(rel @ rpb_w1.T + rpb_b1, 0.0)
    gtbl = (hdn @ rpb_w2.T + rpb_b2).T.astype(np.float32)       # [12, 3969]
    gtbl = np.exp(gtbl, dtype=np.float32)                       # exp(bias)
    gpad = np.zeros((H, TW), np.float32)
    gpad[:, :3969] = gtbl
    gpad = gpad.astype(ml_dtypes.bfloat16)

    bf = ml_dtypes.bfloat16
    wqkvT = np.ascontiguousarray(qkv_w.T).astype(np.float32)    # [768, 2304]
    wvq = np.ascontiguousarray(np.concatenate(
        [wqkvT[:, 2 * C:3 * C], wqkvT[:, 0:C]], axis=1)).astype(bf)
    wkT = wqkvT[:, C:2 * C]
    wprojT = np.ascontiguousarray(proj_w.T).astype(bf)          # [768, 768]
    pbrep = np.ascontiguousarray(
        np.broadcast_to(proj_b.astype(np.float32), (128, C)))
    shared = {"wvq": wvq, "proj_wT": wprojT, "proj_b_rep": pbrep,
              "rpb_tbl": gpad}
    in_maps = []
    for i in range(B):
        m = dict(shared)
        m["xrwk"] = np.ascontiguousarray(np.concatenate(
            [x[i][::-1].T, wkT], axis=1)).astype(bf)
        in_maps.append(m)
    return in_maps


def kernel(x, qkv_w, proj_w, proj_b, rpb_w1, rpb_b1, rpb_w2, rpb_b2,
           _trace=False, _tmpdir=None):
    in_maps = _host_prep(np.asarray(x), np.asarray(qkv_w), np.asarray(proj_w),
                         np.asarray(proj_b), np.asarray(rpb_w1),
                         np.asarray(rpb_b1), np.asarray(rpb_w2),
                         np.asarray(rpb_b2))
    nc = _graph()
    res = run_bass_kernel_spmd(nc, in_maps, core_ids=list(range(B)),
                               trace=_trace, tmpdir=_tmpdir)
    out = np.stack([np.asarray(res.results[i]["out"], dtype=np.float32)
                    for i in range(B)])
    if _trace:
        kernel._last_results = res
    return out
